# revision 41
# baseline (speedup 1.0000x reference)
"""Trainium2 Bass kernel for nn_Decoder_68289980006849 (3-layer transformer decoder).

Strategy: data-parallel over batch (B=8) across 8 NeuronCores; zero collectives.
Per core, the full decoder runs in "T-layout" [feature(partitions), token(free)].

Key optimizations over the straightforward T-layout kernel:
  - weights, K/V/ctx/exp tensors in bf16: halves weight DMA (192->96MB) and
    PE weight-load time; matmul rate is unchanged (1 cycle/row for moving>=256)
  - LayerNorm is algebraically folded into the Q/K projections:
      W @ LN(x) = rstd * (W @ x - m * rowsum(W))
    so the projection matmuls run on the *unnormalized* residual stream
    (ready immediately), a K=1 rank-1 matmul adds the -m*rowsum(W) term in
    PSUM, and the rstd multiply rides the eviction. The LN stats matmuls are
    emitted inside the *previous* phase's residual-update callbacks, with
    x^2 computed incrementally at residual-update time.
  - causal mask applied inside the scores PSUM accumulation group via an
    extra matmul (stationary = NEG*triu, moving = identity) - no separate
    element-wise mask op on the ctx critical path.
  - softmax denominators come free from 64 ones-columns appended to V;
    normalization = one reciprocal + one multiply per head, reading PSUM.
  - FFN software-pipelined: FFN1(e8+1) is emitted before FFN2(e8) so the
    tensor engine never waits on the ReLU evictions; weight tiles
    double-buffered in dedicated pool tags with early DMA prefetch.
  - cross-attention Q-projection runs one chunk ahead of its SDP heads.
  - cross-attn output kept on-chip (no DRAM round trip): c lives in xT,
    x+c in a reused tile for the LN3 stats (reference quirk x = c + f).

Self-contained: only stdlib + numpy + ml_dtypes + the concourse/bass stack.
"""

import os
import numpy as np
import ml_dtypes

import concourse.bass as bass
import concourse.tile as tile
from concourse import bacc, mybir
from concourse.masks import make_identity

# ---- problem constants (hardcoded per contract) ----
B, LD, LE = 8, 512, 512
D, H, DK, F, L, V = 1024, 16, 64, 4096, 3, 32000
M = LD                      # tokens per core
DCH = D // 128              # 8 d-model chunks
FCH = F // 128              # 32 ffn chunks
MCH = M // 128              # 4 token chunks
SQRT_D = 32.0
INV_SQRT_DK = 0.125
EPS = 1e-5
NEG = -1e9
NONES = 64                  # ones-columns appended to V (denominator rows)

P = 128
N = 512
F32 = mybir.dt.float32
F32R = mybir.dt.float32r
BF16 = mybir.dt.bfloat16
AF = mybir.ActivationFunctionType
ALU = mybir.AluOpType

_CACHE = {}
DBG = bool(int(os.environ.get("DECODER_DBG", "0")))


# ----------------------------------------------------------------------------
# Bass program (identical on all 8 cores; data differs via in_maps)
# ----------------------------------------------------------------------------

def _build_nc():
    nc = bacc.Bacc("TRN2", target_bir_lowering=False, debug=False,
                   enable_asserts=False, num_devices=8)

    # inputs (per core)
    x0p = nc.dram_tensor("x0p", [P, DCH, N], BF16, kind="ExternalInput").ap()
    pep = nc.dram_tensor("pep", [P, DCH, N], BF16, kind="ExternalInput").ap()
    encp = nc.dram_tensor("encp", [P, DCH, N], BF16, kind="ExternalInput").ap()
    # projection weights, packed: [l, a, i(q,k,v,o), g, 128, 8, 512], bf16
    wp = nc.dram_tensor("wp", [L, 2, 4, 2, P, DCH, N], BF16, kind="ExternalInput").ap()
    w1p = nc.dram_tensor("w1p", [L, 8, P, DCH, N], BF16, kind="ExternalInput").ap()
    # FFN2 weights packed per f-eighth: [128, j(8), ko(4), 128]
    w2p = nc.dram_tensor("w2p", [L, 8, P, DCH, 4, P], BF16, kind="ExternalInput").ap()
    maskp = nc.dram_tensor("maskp", [P, P], BF16, kind="ExternalInput").ap()
    jscaleb = nc.dram_tensor("jscaleb", [P, P], BF16, kind="ExternalInput").ap()
    vones = nc.dram_tensor("vones", [P, MCH, H, NONES], BF16, kind="ExternalInput").ap()
    # negated row-sums of (wq_self, wk_self, wq_cross) per layer
    wsums = nc.dram_tensor("wsums", [L, 1, 3, D], BF16, kind="ExternalInput").ap()
    ws1s = nc.dram_tensor("ws1s", [L, 1, F], BF16, kind="ExternalInput").ap()
    out = nc.dram_tensor("out", [M, D], F32, kind="ExternalOutput").ap()
    if DBG:
        dbgf = nc.dram_tensor("dbgf", [10, P, N], F32, kind="ExternalOutput").ap()
        dbgb = nc.dram_tensor("dbgb", [12, P, N], BF16, kind="ExternalOutput").ap()

    with tile.TileContext(nc) as tc:
        with tc.tile_pool(name="res", bufs=1) as res, \
             tc.tile_pool(name="wpool", bufs=3) as wpool, \
             tc.tile_pool(name="fpool", bufs=2) as fpool, \
             tc.tile_pool(name="spool", bufs=2) as spool, \
             tc.tile_pool(name="psum", bufs=1, space="PSUM") as psum:

            # ---- resident tiles ----
            xT = res.tile([P, DCH, N], F32R)        # residual stream (fp32)
            xTb = res.tile([P, DCH, N], BF16)       # bf16 twin for matmul reads
            sqT = res.tile([P, DCH, N], BF16)       # x^2 for next LN stats
            encT = res.tile([P, DCH, N], BF16)      # encoder output (transposed)
            Vst = res.tile([P, MCH, H, NONES + 64], BF16)  # ones + V columns
            Jscb = res.tile([P, P], BF16)           # all-(1/D) for LN stats
            maskT = res.tile([P, P], BF16)          # NEG * triu(k=1)
            ident = res.tile([P, P], F32)
            identb = res.tile([P, P], BF16)
            epsc = res.tile([P, 1], F32)
            wsT = res.tile([1, 3, D], BF16)         # per-layer -rowsum(W) rows
            ws1T = res.tile([1, F], BF16)           # -rowsum(ffn_w1) row

            nc.vector.memset(epsc[:], EPS)

            def stats_mm(mean_ps, msq_ps, c, src):
                """Emit the two LN-stats matmuls for chunk c (ride evictions)."""
                nc.tensor.matmul(mean_ps[:], Jscb[:], src[:, c],
                                 start=(c == 0), stop=(c == DCH - 1),
                                 skip_group_check=True)
                nc.tensor.matmul(msq_ps[:], Jscb[:], sqT[:, c],
                                 start=(c == 0), stop=(c == DCH - 1),
                                 skip_group_check=True)

            # ---- x = x0 * sqrt(D) + pe; LN1(l=0) stats ride the init ----
            mean_ps = psum.tile([P, N], F32, tag="st", bufs=2, name="mean_ps")
            msq_ps = psum.tile([P, N], F32, tag="st", bufs=2, name="msq_ps")
            with tc.tile_pool(name="init", bufs=2) as init:
                nc.sync.dma_start(Jscb[:], jscaleb)
                for c in range(DCH):
                    x0c = init.tile([P, N], BF16, tag="x0", name="x0c")
                    pec = init.tile([P, N], BF16, tag="pe", name="pec")
                    nc.sync.dma_start(x0c[:], x0p[:, c])
                    nc.sync.dma_start(pec[:], pep[:, c])
                    nc.vector.scalar_tensor_tensor(
                        xT[:, c], x0c[:], SQRT_D, pec[:], ALU.mult, ALU.add)
                    nc.scalar.activation(xTb[:, c], xT[:, c].bitcast(F32),
                                         AF.Copy)
                    nc.scalar.activation(sqT[:, c], xTb[:, c], AF.Square)
                    stats_mm(mean_ps, msq_ps, c, xTb)

            if DBG:
                nc.sync.dma_start(dbgf[0], xT[:, 0].bitcast(F32))
                nc.sync.dma_start(dbgb[0], xTb[:, 0])
                nc.sync.dma_start(dbgb[1], sqT[:, 0])

            def preload_qk(l):
                t = {}
                for key, a, i, g in (("q", 0, 0, 0), ("q", 0, 0, 1),
                                     ("k", 0, 1, 0)):
                    wt = wpool.tile([P, DCH, N], BF16, tag="wt", name="wt")
                    nc.sync.dma_start(wt[:, 0:4], wp[l, a, i, g][:, 0:4])
                    nc.sync.dma_start(wt[:, 4:8], wp[l, a, i, g][:, 4:8])
                    t[(key, g)] = wt
                return t

            # layer-0 Q/K weights queue BEFORE encp/vones: the projections
            # need them ~40us earlier than the encoder tensors are read
            _pre = preload_qk(0)

            nc.sync.dma_start(encT[:], encp)
            nc.sync.dma_start(maskT[:], maskp)
            nc.sync.dma_start(Vst[:, :, :, 0:NONES], vones)
            make_identity(nc, ident[:])
            nc.vector.tensor_copy(identb[:], ident[:])

            _dbg_state = {}
            work = tc.alloc_tile_pool(name="work", bufs=1)
            hT = work.tile([P, DCH, N], BF16, tag="hT")     # LN apply output
            ctxT = work.tile([P, DCH, N], BF16, tag="ctxT")
            qT = work.tile([P, DCH, N], BF16, tag="qT")     # also x+c for LN3
            kT = work.tile([P, DCH, N], BF16, tag="kT")

            # ---- helpers ----
            def ln_chain(mean_ps, msq_ps):
                """PSUM stats -> (rstd bcast [P,N], mean row [1,N])."""
                m2 = spool.tile([P, N], F32, tag="stt", name="m2")
                nc.scalar.activation(m2[:], mean_ps[:], AF.Square)
                var = spool.tile([P, N], F32, tag="stt", name="var")
                nc.vector.tensor_tensor(var[:], msq_ps[:], m2[:], op=ALU.subtract)
                sd = spool.tile([P, N], F32, tag="stt", name="sd")
                nc.scalar.activation(sd[:], var[:], AF.Sqrt, bias=epsc[:])
                rstd = spool.tile([P, N], F32, tag="stt", name="rstd")
                nc.vector.reciprocal_approx_fast(rstd[:], sd[:])
                mrow = spool.tile([1, N], BF16, tag="mrow", name="mrow")
                nc.vector.tensor_copy(mrow[:], mean_ps[0:1, :])
                if DBG and not _dbg_state.get("chain"):
                    _dbg_state["chain"] = True
                    ms = spool.tile([P, N], F32, tag="dbgms", name="dbgms")
                    nc.scalar.activation(ms[:], mean_ps[:], AF.Copy)
                    nc.sync.dma_start(dbgf[1], ms[:])
                    nc.sync.dma_start(dbgf[2], rstd[:])
                return rstd, mrow

            def ln_apply(dst, src, mean_ps, rstd, src_bf16=False):
                """dst[:] = (src - mean) * rstd, cast to dst dtype."""
                for c in range(DCH):
                    cen = spool.tile([P, N], F32, tag="cen", name="cen")
                    nc.vector.tensor_tensor(
                        cen[:], src[:, c] if src_bf16 else src[:, c].bitcast(F32),
                        mean_ps[:], op=ALU.subtract)
                    nc.vector.tensor_tensor(dst[:, c], cen[:], rstd[:],
                                            op=ALU.mult)

            def proj_ln(w_groups, ws_idx, rstd, mrow, dst, pre=None):
                """dst[:, j] = rstd * (W @ x - m*rowsum(W)) per chunk, bf16."""
                for g in range(2):
                    if pre and g in pre:
                        wt = pre.pop(g)
                    else:
                        wt = wpool.tile([P, DCH, N], BF16, tag="wt", name="wt")
                        nc.sync.dma_start(wt[:, 0:4], w_groups[g][:, 0:4])
                        nc.sync.dma_start(wt[:, 4:8], w_groups[g][:, 4:8])
                    for jj in range(4):
                        j = g * 4 + jj
                        ps = psum.tile([P, N], F32, tag="mm", bufs=2, name="ps_mm")
                        for k in range(DCH):
                            nc.tensor.matmul(ps[:], wt[:, k, jj * P:(jj + 1) * P],
                                             xTb[:, k], start=(k == 0), stop=False,
                                             skip_group_check=True)
                        nc.tensor.matmul(ps[:], wsT[0:1, ws_idx, j * P:(j + 1) * P],
                                         mrow[:], start=False, stop=True,
                                         skip_group_check=True)
                        nc.vector.tensor_tensor(dst[:, j], ps[:], rstd[:],
                                                op=ALU.mult)

            def proj(rhs, w_groups, evict):
                """out[j] = sum_k W[k, j-chunk].T @ rhs[k]."""
                for g in range(2):
                    wt = wpool.tile([P, DCH, N], BF16, tag="wt", name="wt")
                    nc.sync.dma_start(wt[:, 0:4], w_groups[g][:, 0:4])
                    nc.sync.dma_start(wt[:, 4:8], w_groups[g][:, 4:8])
                    for jj in range(4):
                        j = g * 4 + jj
                        ps = psum.tile([P, N], F32, tag="mm", bufs=2, name="ps_mm")
                        for k in range(DCH):
                            nc.tensor.matmul(ps[:], wt[:, k, jj * P:(jj + 1) * P],
                                             rhs[:, k], start=(k == 0),
                                             stop=(k == DCH - 1))
                        evict(j, ps)

            def v_proj(w_groups):
                """V_nat[tk, dv] -> Vst[:, t, h, :64] slices (from hT)."""
                for g in range(2):  # dv halves (heads g*8..g*8+7)
                    wt = wpool.tile([P, DCH, N], BF16, tag="wt", name="wt")
                    nc.sync.dma_start(wt[:], w_groups[g])
                    for t in range(MCH):
                        ps = psum.tile([P, N], F32, tag="mm", bufs=2, name="ps_v")
                        for k in range(DCH):
                            nc.tensor.matmul(ps[:], hT[:, k, t * P:(t + 1) * P],
                                             wt[:, k], start=(k == 0),
                                             stop=(k == DCH - 1))
                        nc.scalar.activation(
                            Vst[:, t, g * 8:(g + 1) * 8, NONES:NONES + 64],
                            ps[:].rearrange("p (h d) -> p h d", d=64), AF.Copy)

            def sdp_scores(q_t, k_t, h, is_self):
                """Scores_T -> exp for one head; ctx is emitted one head later
                (software pipeline) so the exp latency never stalls the PE."""
                hc, off = h // 2, 64 * (h % 2)
                exps = []
                for c in range(MCH):
                    cs = c * P if is_self else 0
                    sc = psum.tile([P, N], F32, tag="sc", bufs=2, name="sc")
                    nc.tensor.matmul(sc[:, cs:],
                                     k_t[off:off + 64, hc, c * P:(c + 1) * P],
                                     q_t[off:off + 64, hc, cs:],
                                     start=True, stop=not is_self,
                                     skip_group_check=True)
                    if is_self:
                        # causal mask for the diagonal block, in PSUM
                        nc.tensor.matmul(sc[:, cs:cs + P], maskT[:], identb[:],
                                         start=False, stop=True,
                                         skip_group_check=True)
                    ex = spool.tile([P, N], BF16, tag="exp", bufs=24, name="ex")
                    nc.scalar.activation(ex[:, cs:], sc[:, cs:], AF.Exp,
                                         scale=INV_SQRT_DK)
                    exps.append(ex)
                return exps

            def sdp_ctx(h, exps, is_self):
                hc, off = h // 2, 64 * (h % 2)
                ctx = psum.tile([P, N], F32, tag="ctx", bufs=2, name="ctx")
                for c in range(MCH):
                    cs = c * P if is_self else 0
                    nc.tensor.matmul(ctx[:, cs:], Vst[:, c, h, :],
                                     exps[c][:, cs:], start=(c == 0),
                                     stop=(c == MCH - 1), skip_group_check=True)
                # ones-columns come FIRST in Vst so the denominators land at
                # PSUM partition base 0: reciprocal_approx_fast (custom DVE
                # op) mishandles nonzero partition offsets on PSUM inputs on
                # hardware (reads base 0 regardless; CoreSim models it right).
                rec = spool.tile([NONES, N], F32, tag="rec", name="rec")
                nc.vector.reciprocal_approx_fast(rec[:], ctx[0:NONES, :])
                nc.vector.tensor_tensor(ctxT[off:off + 64, hc, :],
                                        ctx[NONES:NONES + 64, :],
                                        rec[:], op=ALU.mult)

            def attention(q_t, k_t, is_self, wo_groups, out_evict, fillers=None):
                """`fillers` maps head-index -> callables emitting independent
                PE work (the next attention's K/V projections)."""
                pend = None
                for h in range(H):
                    if fillers:
                        for cb in fillers.get(h, ()):
                            cb()
                    exps = sdp_scores(q_t, k_t, h, is_self)
                    if pend is not None:
                        sdp_ctx(pend, pend_exps, is_self)
                    pend, pend_exps = h, exps
                sdp_ctx(pend, pend_exps, is_self)
                if fillers:
                    for cb in fillers.get(H, ()):
                        cb()
                proj(ctxT, wo_groups, out_evict)

            for l in range(L):
                def wgrp(a, i):
                    return [wp[l, a, i, g] for g in range(2)]

                # ---- self-attention ----
                nc.sync.dma_start(wsT[:], wsums[l])
                nc.sync.dma_start(ws1T[:], ws1s[l])
                rstd1, mrow1 = ln_chain(mean_ps, msq_ps)
                proj_ln(wgrp(0, 0), 0, rstd1, mrow1, qT,
                        pre={0: _pre.pop(("q", 0)), 1: _pre.pop(("q", 1))})
                proj_ln(wgrp(0, 1), 1, rstd1, mrow1, kT,
                        pre={0: _pre.pop(("k", 0))})
                ln_apply(hT, xT, mean_ps, rstd1)
                v_proj(wgrp(0, 2))
                if DBG and l == 0:
                    nc.sync.dma_start(dbgb[2], qT[:, 0])
                    nc.sync.dma_start(dbgb[3], kT[:, 0])
                    nc.sync.dma_start(dbgb[4], hT[:, 0])
                    nc.sync.dma_start(dbgb[6][:, 0:128], Vst[:, 0, 0, :])
                    nc.sync.dma_start(dbgb[7][:, 0:128], Vst[:, 0, 8, :])

                # LN2 stats ride the self-attn residual evictions
                mean_ps2 = psum.tile([P, N], F32, tag="st", bufs=2, name="mean2")
                msq_ps2 = psum.tile([P, N], F32, tag="st", bufs=2, name="msq2")

                def ev_self_o(j, ps):
                    nc.vector.tensor_tensor(xT[:, j], ps[:], xT[:, j].bitcast(F32),
                                            op=ALU.add)
                    nc.scalar.activation(xTb[:, j], xT[:, j].bitcast(F32),
                                         AF.Copy)
                    nc.scalar.activation(sqT[:, j], xTb[:, j], AF.Square)
                    stats_mm(mean_ps2, msq_ps2, j, xTb)

                # cross-attn K/V projections fill the self-attention SDP
                ck_state = {}
                wkx, wvx = wgrp(1, 1), wgrp(1, 2)

                def ck_dma(g):
                    def f():
                        wt = wpool.tile([P, DCH, N], BF16, tag="wt", name="wt")
                        nc.sync.dma_start(wt[:, 0:4], wkx[g][:, 0:4])
                        nc.sync.dma_start(wt[:, 4:8], wkx[g][:, 4:8])
                        ck_state["k"] = wt
                    return f

                def ck_chunk(g, jj):
                    def f():
                        wt = ck_state["k"]
                        j = g * 4 + jj
                        ps = psum.tile([P, N], F32, tag="mm", bufs=2, name="ps_ck")
                        for k in range(DCH):
                            nc.tensor.matmul(ps[:], wt[:, k, jj * P:(jj + 1) * P],
                                             encT[:, k], start=(k == 0),
                                             stop=(k == DCH - 1))
                        nc.vector.tensor_copy(kT[:, j], ps[:])
                    return f

                def cv_dma(g):
                    def f():
                        wt = wpool.tile([P, DCH, N], BF16, tag="wt", name="wt")
                        nc.sync.dma_start(wt[:], wvx[g])
                        ck_state["v"] = wt
                    return f

                def cv_chunk(g, t):
                    def f():
                        wt = ck_state["v"]
                        ps = psum.tile([P, N], F32, tag="mm", bufs=2, name="ps_cv")
                        for k in range(DCH):
                            nc.tensor.matmul(ps[:], encT[:, k, t * P:(t + 1) * P],
                                             wt[:, k], start=(k == 0),
                                             stop=(k == DCH - 1))
                        nc.vector.tensor_copy(
                            Vst[:, t, g * 8:(g + 1) * 8, NONES:NONES + 64],
                            ps[:].rearrange("p (h d) -> p h d", d=64))
                    return f

                fillers = {
                    0: [ck_dma(0)],
                    2: [ck_chunk(0, 0)], 4: [ck_chunk(0, 1)],
                    6: [ck_chunk(0, 2)], 8: [ck_chunk(0, 3), ck_dma(1)],
                    10: [ck_chunk(1, 0), cv_dma(0)],
                    12: [ck_chunk(1, 1), cv_chunk(0, 0)],
                    13: [cv_chunk(0, 1)],
                    14: [ck_chunk(1, 2), cv_chunk(0, 2)],
                    15: [cv_chunk(0, 3)],
                    16: [ck_chunk(1, 3), cv_dma(1),
                         cv_chunk(1, 0), cv_chunk(1, 1),
                         cv_chunk(1, 2), cv_chunk(1, 3)],
                }
                attention(qT, kT, True, wgrp(0, 3), ev_self_o, fillers=fillers)
                if DBG and l == 0:
                    nc.sync.dma_start(dbgf[4], xT[:, 0].bitcast(F32))
                    nc.sync.dma_start(dbgb[8], ctxT[:, 0])

                # ---- cross-attention ----
                # Q-projection runs one chunk ahead of its SDP head pairs.
                rstd2, mrow2 = ln_chain(mean_ps2, msq_ps2)
                wqx = wgrp(1, 0)

                def qx_chunk(g, jj, wt):
                    j = g * 4 + jj
                    ps = psum.tile([P, N], F32, tag="mm", bufs=2, name="ps_cq")
                    for k in range(DCH):
                        nc.tensor.matmul(ps[:], wt[:, k, jj * P:(jj + 1) * P],
                                         xTb[:, k], start=(k == 0), stop=False,
                                         skip_group_check=True)
                    nc.tensor.matmul(ps[:], wsT[0:1, 2, j * P:(j + 1) * P],
                                     mrow2[:], start=False, stop=True,
                                     skip_group_check=True)
                    nc.vector.tensor_tensor(qT[:, j], ps[:], rstd2[:],
                                            op=ALU.mult)

                wts = {}
                for g in range(2):
                    wt = wpool.tile([P, DCH, N], BF16, tag="wt", name="wt")
                    nc.sync.dma_start(wt[:, 0:4], wqx[g][:, 0:4])
                    nc.sync.dma_start(wt[:, 4:8], wqx[g][:, 4:8])
                    wts[g] = wt
                xexps = {}
                for j in range(DCH + 2):
                    if j < DCH:
                        qx_chunk(j // 4, j % 4, wts[j // 4])
                    if 1 <= j <= DCH:
                        for h in (2 * (j - 1), 2 * (j - 1) + 1):
                            xexps[h] = sdp_scores(qT, kT, h, False)
                    if j >= 3:
                        for h in (2 * (j - 3), 2 * (j - 3) + 1):
                            sdp_ctx(h, xexps.pop(h), False)
                for h in (14, 15):
                    sdp_ctx(h, xexps.pop(h), False)

                # prefetch first FFN weight tiles during the cross O-proj
                w1t = {0: fpool.tile([P, DCH, N], BF16, tag="w1", name="w1t")}
                nc.sync.dma_start(w1t[0][:], w1p[l, 0])
                w2t = {0: fpool.tile([P, DCH, 4, P], BF16, tag="w2", name="w2t")}
                nc.sync.dma_start(w2t[0][:], w2p[l, 0])

                # LN3 stats (over x+c, kept in qT) ride the cross evictions
                mean_ps3 = psum.tile([P, N], F32, tag="st", bufs=2, name="mean3")
                msq_ps3 = psum.tile([P, N], F32, tag="st", bufs=2, name="msq3")

                def ev_cross_o(j, ps):
                    # qT <- x + c (for LN3); xT <- c (reference residual quirk)
                    nc.vector.tensor_tensor(qT[:, j], ps[:], xT[:, j].bitcast(F32),
                                            op=ALU.add)
                    nc.scalar.activation(xT[:, j].bitcast(F32), ps[:], AF.Copy)
                    nc.scalar.activation(sqT[:, j], qT[:, j], AF.Square)
                    stats_mm(mean_ps3, msq_ps3, j, qT)
                proj(ctxT, wgrp(1, 3), ev_cross_o)
                if DBG and l == 0:
                    nc.sync.dma_start(dbgb[9], qT[:, 0])
                    nc.sync.dma_start(dbgf[5], xT[:, 0].bitcast(F32))

                # ---- FFN (8 f-eighths, software-pipelined) ----
                # FFN1 uses the LN-fixup scheme on sT (in qT) directly:
                # relu(rstd * (W1 @ sT - m * rowsum(W1)))
                rstd3, mrow3 = ln_chain(mean_ps3, msq_ps3)

                if l + 1 < L:
                    mean_psN = psum.tile([P, N], F32, tag="st", bufs=2, name="meanN")
                    msq_psN = psum.tile([P, N], F32, tag="st", bufs=2, name="msqN")

                u8s = {}

                def ffn1(e8):
                    u8 = spool.tile([P, 4, N], BF16, tag="uT", name="u8")
                    wt = w1t.pop(e8)
                    for jj in range(4):
                        f = e8 * 4 + jj
                        ps = psum.tile([P, N], F32, tag="mm", bufs=2, name="ps_f1")
                        for k in range(DCH):
                            nc.tensor.matmul(ps[:], wt[:, k, jj * P:(jj + 1) * P],
                                             qT[:, k], start=(k == 0), stop=False,
                                             skip_group_check=True)
                        nc.tensor.matmul(ps[:], ws1T[0:1, f * P:(f + 1) * P],
                                         mrow3[:], start=False, stop=True,
                                         skip_group_check=True)
                        t1 = spool.tile([P, N], F32, tag="cen", name="t1")
                        nc.vector.tensor_tensor(t1[:], ps[:], rstd3[:],
                                                op=ALU.mult)
                        nc.scalar.activation(u8[:, jj], t1[:], AF.Relu)
                    u8s[e8] = u8

                def ffn2(e8):
                    u8 = u8s.pop(e8)
                    wt2 = w2t.pop(e8)
                    for j in range(DCH):
                        ps = psum.tile([P, N], F32, tag="mm", bufs=2, name="ps_f2")
                        for k in range(4):
                            nc.tensor.matmul(ps[:], wt2[:, j, k], u8[:, k],
                                             start=(k == 0), stop=(k == 3))
                        # for e8 == 0 this is x = cross_out + ffn_part0
                        # (xT already holds c, the reference residual quirk)
                        nc.vector.tensor_tensor(xT[:, j], ps[:],
                                                xT[:, j].bitcast(F32),
                                                op=ALU.add)
                        if e8 == 7 and l + 1 < L:
                            nc.scalar.activation(xTb[:, j],
                                                 xT[:, j].bitcast(F32), AF.Copy)
                            nc.scalar.activation(sqT[:, j], xTb[:, j], AF.Square)
                            stats_mm(mean_psN, msq_psN, j, xTb)
                        if e8 == 7 and l + 1 == L:
                            for m in range(MCH):
                                pst = psum.tile([P, N], F32, tag="sc", bufs=2,
                                                name="pst")
                                nc.tensor.transpose(
                                    pst[:, 0:P],
                                    xT[:, j, m * P:(m + 1) * P].bitcast(F32),
                                    ident[:])
                                tsb = spool.tile([P, P], F32, tag="osb", bufs=2,
                                                 name="tsb")
                                nc.scalar.activation(tsb[:], pst[:, 0:P], AF.Copy)
                                nc.sync.dma_start(
                                    out[m * P:(m + 1) * P, j * P:(j + 1) * P],
                                    tsb[:])

                w1t[1] = fpool.tile([P, DCH, N], BF16, tag="w1", name="w1t")
                nc.sync.dma_start(w1t[1][:], w1p[l, 1])
                ffn1(0)
                for e8 in range(8):
                    if e8 == 3 and l + 1 < L:
                        _pre.update(preload_qk(l + 1))
                    if e8 + 1 < 8:
                        w2t[e8 + 1] = fpool.tile([P, DCH, 4, P], BF16,
                                                 tag="w2", name="w2t")
                        nc.sync.dma_start(w2t[e8 + 1][:], w2p[l, e8 + 1])
                        if e8 + 2 < 8:
                            w1t[e8 + 2] = fpool.tile([P, DCH, N], BF16,
                                                     tag="w1", name="w1t")
                            nc.sync.dma_start(w1t[e8 + 2][:], w1p[l, e8 + 2])
                        ffn1(e8 + 1)
                    ffn2(e8)
                if DBG:
                    nc.sync.dma_start(dbgf[6 + l], xT[:, 0].bitcast(F32))

                if l + 1 < L:
                    mean_ps, msq_ps = mean_psN, msq_psN

            work.release()

    nc.compile()
    return nc


# ----------------------------------------------------------------------------
# host-side packing
# ----------------------------------------------------------------------------

def _pack_T(aT):
    """[1024, C] (feature-major) -> tile image [128, 8, C]."""
    d, c = aT.shape
    return np.ascontiguousarray(aT.reshape(DCH, P, c).transpose(1, 0, 2))


def _pack_proj(w):
    """w [Dout, Din] (as in y = x @ w.T) -> [2, 128, 8, 512] group tile images."""
    wT = w.T  # [Din, Dout]
    return np.stack([_pack_T(wT[:, g * N:(g + 1) * N]) for g in range(2)])


def _prep(inputs):
    BH = ml_dtypes.bfloat16
    dec_inputs = np.asarray(inputs["dec_inputs"])
    self_mask = np.asarray(inputs["self_mask"])
    enc_output = np.asarray(inputs["enc_output"], dtype=np.float32)
    encoder_mask = np.asarray(inputs["encoder_mask"])
    embed = np.asarray(inputs["embed"], dtype=np.float32)
    pe = np.asarray(inputs["pe"], dtype=np.float32)
    wq, wk, wv, wo = (np.asarray(inputs[k], np.float32) for k in ("wq", "wk", "wv", "wo"))
    w1, w2 = np.asarray(inputs["ffn_w1"], np.float32), np.asarray(inputs["ffn_w2"], np.float32)

    # structural assumptions baked into the kernel
    causal_ref = np.triu(np.ones((LD, LD), bool), k=1)
    assert all(np.array_equal(self_mask[b], causal_ref) for b in range(B)), \
        "kernel assumes causal self mask"
    assert not encoder_mask.any(), "kernel assumes no encoder mask"
    for k in ("bq", "bk", "bv", "bo", "ffn_b1", "ffn_b2", "ln_b"):
        assert not np.asarray(inputs[k]).any(), f"kernel assumes zero {k}"
    assert np.all(np.asarray(inputs["ln_g"]) == 1.0), "kernel assumes unit ln gains"

    # shared (weight) arrays
    wp = np.empty((L, 2, 4, 2, P, DCH, N), BH)
    for l in range(L):
        for a in range(2):
            for i, w in enumerate((wq, wk, wv, wo)):
                wp[l, a, i] = _pack_proj(w[l, a]).astype(BH)
    w1p = np.empty((L, 8, P, DCH, N), BH)
    w2p = np.empty((L, 8, P, DCH, 4, P), BH)
    for l in range(L):
        w1T = w1[l].T  # [1024, 4096]
        for g in range(8):
            w1p[l, g] = _pack_T(w1T[:, g * N:(g + 1) * N]).astype(BH)
        w2T = w2[l].T  # [4096, 1024]
        blk = w2T.reshape(8, 4, P, DCH, P)
        w2p[l] = np.ascontiguousarray(blk.transpose(0, 2, 3, 1, 4)).astype(BH)

    # negated row-sums for the LN fixup matmuls: q_self, k_self, q_cross
    wsums = np.empty((L, 1, 3, D), np.float32)
    ws1s = np.empty((L, 1, F), np.float32)
    for l in range(L):
        wsums[l, 0, 0] = -wq[l, 0].sum(axis=1)
        wsums[l, 0, 1] = -wk[l, 0].sum(axis=1)
        wsums[l, 0, 2] = -wq[l, 1].sum(axis=1)
        ws1s[l, 0] = -w1[l].sum(axis=1)

    pep = _pack_T(pe.T).astype(BH)
    maskp = (NEG * np.triu(np.ones((P, P), np.float32), 1)).astype(BH)
    jscaleb = np.full((P, P), 1.0 / D, BH)
    vones = np.ones((P, MCH, H, NONES), BH)

    shared = dict(wp=wp, w1p=w1p, w2p=w2p, pep=pep, maskp=maskp,
                  jscaleb=jscaleb, vones=vones,
                  wsums=wsums.astype(BH), ws1s=ws1s.astype(BH))
    in_maps = []
    for b in range(B):
        x0 = embed[dec_inputs[b]]          # [512, 1024]
        m = dict(shared)
        m["x0p"] = _pack_T(np.ascontiguousarray(x0.T)).astype(BH)
        m["encp"] = _pack_T(np.ascontiguousarray(enc_output[b].T)).astype(BH)
        in_maps.append(m)
    return in_maps


def kernel(**inputs):
    if "nc" not in _CACHE:
        _CACHE["nc"] = _build_nc()
    nc = _CACHE["nc"]
    in_maps = _prep(inputs)

    from concourse import bass_utils
    trace = bool(int(os.environ.get("DECODER_TRACE", "0")))
    res = bass_utils.run_bass_kernel_spmd(
        nc, in_maps, core_ids=list(range(B)), trace=trace)
    _CACHE["last_result"] = res
    return np.stack([res.results[b]["out"] for b in range(B)]).astype(np.float32)


# revision 42
# speedup vs baseline: 1.0447x; 1.0447x over previous
"""Trainium2 Bass kernel for nn_Decoder_68289980006849 (3-layer transformer decoder).

Strategy: data-parallel over batch (B=8) across 8 NeuronCores; zero collectives.
Per core, the full decoder runs in "T-layout" [feature(partitions), token(free)].

Key optimizations over the straightforward T-layout kernel:
  - weights, K/V/ctx/exp tensors in bf16: halves weight DMA (192->96MB) and
    PE weight-load time; matmul rate is unchanged (1 cycle/row for moving>=256)
  - LayerNorm is algebraically folded into the Q/K projections:
      W @ LN(x) = rstd * (W @ x - m * rowsum(W))
    so the projection matmuls run on the *unnormalized* residual stream
    (ready immediately), a K=1 rank-1 matmul adds the -m*rowsum(W) term in
    PSUM, and the rstd multiply rides the eviction. The LN stats matmuls are
    emitted inside the *previous* phase's residual-update callbacks, with
    x^2 computed incrementally at residual-update time.
  - causal mask applied inside the scores PSUM accumulation group via an
    extra matmul (stationary = NEG*triu, moving = identity) - no separate
    element-wise mask op on the ctx critical path.
  - softmax denominators come free from 64 ones-columns appended to V;
    normalization = one reciprocal + one multiply per head, reading PSUM.
  - FFN software-pipelined: FFN1(e8+1) is emitted before FFN2(e8) so the
    tensor engine never waits on the ReLU evictions; weight tiles
    double-buffered in dedicated pool tags with early DMA prefetch.
  - cross-attention Q-projection runs one chunk ahead of its SDP heads.
  - cross-attn output kept on-chip (no DRAM round trip): c lives in xT,
    x+c in a reused tile for the LN3 stats (reference quirk x = c + f).

Self-contained: only stdlib + numpy + ml_dtypes + the concourse/bass stack.
"""

import os
import numpy as np
import ml_dtypes

import concourse.bass as bass
import concourse.tile as tile
from concourse import bacc, mybir
from concourse.masks import make_identity

# ---- problem constants (hardcoded per contract) ----
B, LD, LE = 8, 512, 512
D, H, DK, F, L, V = 1024, 16, 64, 4096, 3, 32000
M = LD                      # tokens per core
DCH = D // 128              # 8 d-model chunks
FCH = F // 128              # 32 ffn chunks
MCH = M // 128              # 4 token chunks
SQRT_D = 32.0
INV_SQRT_DK = 0.125
EPS = 1e-5
NEG = -1e9
NONES = 64                  # ones-columns appended to V (denominator rows)

P = 128
N = 512
F32 = mybir.dt.float32
F32R = mybir.dt.float32r
BF16 = mybir.dt.bfloat16
AF = mybir.ActivationFunctionType
ALU = mybir.AluOpType

_CACHE = {}
DBG = bool(int(os.environ.get("DECODER_DBG", "0")))


# ----------------------------------------------------------------------------
# Bass program (identical on all 8 cores; data differs via in_maps)
# ----------------------------------------------------------------------------

def _build_nc():
    nc = bacc.Bacc("TRN2", target_bir_lowering=False, debug=False,
                   enable_asserts=False, num_devices=8)

    # inputs (per core)
    x0p = nc.dram_tensor("x0p", [P, DCH, N], BF16, kind="ExternalInput").ap()
    pep = nc.dram_tensor("pep", [P, DCH, N], BF16, kind="ExternalInput").ap()
    encp = nc.dram_tensor("encp", [P, DCH, N], BF16, kind="ExternalInput").ap()
    # projection weights, packed: [l, a, i(q,k,v,o), g, 128, 8, 512], bf16
    wp = nc.dram_tensor("wp", [L, 2, 4, 2, P, DCH, N], BF16, kind="ExternalInput").ap()
    w1p = nc.dram_tensor("w1p", [L, 8, P, DCH, N], BF16, kind="ExternalInput").ap()
    # FFN2 weights packed per f-eighth: [128, j(8), ko(4), 128]
    w2p = nc.dram_tensor("w2p", [L, 8, P, DCH, 4, P], BF16, kind="ExternalInput").ap()
    maskp = nc.dram_tensor("maskp", [P, P], BF16, kind="ExternalInput").ap()
    jscaleb = nc.dram_tensor("jscaleb", [P, P], BF16, kind="ExternalInput").ap()
    vones = nc.dram_tensor("vones", [P, MCH, H, NONES], BF16, kind="ExternalInput").ap()
    # negated row-sums of (wq_self, wk_self, wq_cross) per layer
    wsums = nc.dram_tensor("wsums", [L, 1, 3, D], BF16, kind="ExternalInput").ap()
    ws1s = nc.dram_tensor("ws1s", [L, 1, F], BF16, kind="ExternalInput").ap()
    out = nc.dram_tensor("out", [M, D], F32, kind="ExternalOutput").ap()
    if DBG:
        dbgf = nc.dram_tensor("dbgf", [10, P, N], F32, kind="ExternalOutput").ap()
        dbgb = nc.dram_tensor("dbgb", [12, P, N], BF16, kind="ExternalOutput").ap()

    with tile.TileContext(nc) as tc:
        with tc.tile_pool(name="res", bufs=1) as res, \
             tc.tile_pool(name="wpool", bufs=3) as wpool, \
             tc.tile_pool(name="fpool", bufs=2) as fpool, \
             tc.tile_pool(name="spool", bufs=2) as spool, \
             tc.tile_pool(name="psum", bufs=1, space="PSUM") as psum:

            # ---- resident tiles ----
            xT = res.tile([P, DCH, N], F32R)        # residual stream (fp32)
            xTb = res.tile([P, DCH, N], BF16)       # bf16 twin for matmul reads
            sqT = res.tile([P, DCH, N], BF16)       # x^2 for next LN stats
            encT = res.tile([P, DCH, N], BF16)      # encoder output (transposed)
            Vst = res.tile([P, MCH, H, NONES + 64], BF16)  # ones + V columns
            Jscb = res.tile([P, P], BF16)           # all-(1/D) for LN stats
            maskT = res.tile([P, P], BF16)          # NEG * triu(k=1)
            ident = res.tile([P, P], F32)
            identb = res.tile([P, P], BF16)
            epsc = res.tile([P, 1], F32)
            wsT = res.tile([1, 3, D], BF16)         # per-layer -rowsum(W) rows
            ws1T = res.tile([1, F], BF16)           # -rowsum(ffn_w1) row

            nc.vector.memset(epsc[:], EPS)

            def stats_mm(mean_ps, msq_ps, c, src):
                """Emit the two LN-stats matmuls for chunk c (ride evictions)."""
                nc.tensor.matmul(mean_ps[:], Jscb[:], src[:, c],
                                 start=(c == 0), stop=(c == DCH - 1),
                                 skip_group_check=True)
                nc.tensor.matmul(msq_ps[:], Jscb[:], sqT[:, c],
                                 start=(c == 0), stop=(c == DCH - 1),
                                 skip_group_check=True)

            # ---- x = x0 * sqrt(D) + pe; LN1(l=0) stats ride the init ----
            mean_ps = psum.tile([P, N], F32, tag="st", bufs=2, name="mean_ps")
            msq_ps = psum.tile([P, N], F32, tag="st", bufs=2, name="msq_ps")
            with tc.tile_pool(name="init", bufs=2) as init:
                nc.sync.dma_start(Jscb[:], jscaleb)
                for c in range(DCH):
                    x0c = init.tile([P, N], BF16, tag="x0", name="x0c")
                    pec = init.tile([P, N], BF16, tag="pe", name="pec")
                    nc.sync.dma_start(x0c[:], x0p[:, c])
                    nc.sync.dma_start(pec[:], pep[:, c])
                    nc.vector.scalar_tensor_tensor(
                        xT[:, c], x0c[:], SQRT_D, pec[:], ALU.mult, ALU.add)
                    nc.scalar.activation(xTb[:, c], xT[:, c].bitcast(F32),
                                         AF.Copy)
                    nc.scalar.activation(sqT[:, c], xTb[:, c], AF.Square)
                    stats_mm(mean_ps, msq_ps, c, xTb)

            if DBG:
                nc.sync.dma_start(dbgf[0], xT[:, 0].bitcast(F32))
                nc.sync.dma_start(dbgb[0], xTb[:, 0])
                nc.sync.dma_start(dbgb[1], sqT[:, 0])

            # needed from the first SDP phase onwards, not at start
            nc.sync.dma_start(encT[:], encp)
            nc.sync.dma_start(maskT[:], maskp)
            nc.sync.dma_start(Vst[:, :, :, 0:NONES], vones)
            make_identity(nc, ident[:])
            nc.vector.tensor_copy(identb[:], ident[:])

            def preload_qk(l):
                t = {}
                for key, a, i, g in (("q", 0, 0, 0), ("q", 0, 0, 1),
                                     ("k", 0, 1, 0)):
                    wt = wpool.tile([P, DCH, N], BF16, tag="wt", name="wt")
                    nc.sync.dma_start(wt[:, 0:4], wp[l, a, i, g][:, 0:4])
                    nc.sync.dma_start(wt[:, 4:8], wp[l, a, i, g][:, 4:8])
                    t[(key, g)] = wt
                return t

            _pre = preload_qk(0)

            _dbg_state = {}
            work = tc.alloc_tile_pool(name="work", bufs=1)
            hT = work.tile([P, DCH, N], BF16, tag="hT")     # LN apply output
            ctxT = work.tile([P, DCH, N], BF16, tag="ctxT")
            qT = work.tile([P, DCH, N], BF16, tag="qT")     # also x+c for LN3
            kT = work.tile([P, DCH, N], BF16, tag="kT")

            # ---- helpers ----
            def ln_chain(mean_ps, msq_ps):
                """PSUM stats -> (rstd bcast [P,N], mean row [1,N])."""
                m2 = spool.tile([P, N], F32, tag="stt", name="m2")
                nc.scalar.activation(m2[:], mean_ps[:], AF.Square)
                var = spool.tile([P, N], F32, tag="stt", name="var")
                nc.vector.tensor_tensor(var[:], msq_ps[:], m2[:], op=ALU.subtract)
                sd = spool.tile([P, N], F32, tag="stt", name="sd")
                nc.scalar.activation(sd[:], var[:], AF.Sqrt, bias=epsc[:])
                rstd = spool.tile([P, N], F32, tag="stt", name="rstd")
                nc.vector.reciprocal_approx_fast(rstd[:], sd[:])
                mrow = spool.tile([1, N], BF16, tag="mrow", name="mrow")
                nc.vector.tensor_copy(mrow[:], mean_ps[0:1, :])
                if DBG and not _dbg_state.get("chain"):
                    _dbg_state["chain"] = True
                    ms = spool.tile([P, N], F32, tag="dbgms", name="dbgms")
                    nc.scalar.activation(ms[:], mean_ps[:], AF.Copy)
                    nc.sync.dma_start(dbgf[1], ms[:])
                    nc.sync.dma_start(dbgf[2], rstd[:])
                return rstd, mrow

            def ln_apply(dst, src, mean_ps, rstd, src_bf16=False):
                """dst[:] = (src - mean) * rstd, cast to dst dtype."""
                for c in range(DCH):
                    cen = spool.tile([P, N], F32, tag="cen", name="cen")
                    nc.vector.tensor_tensor(
                        cen[:], src[:, c] if src_bf16 else src[:, c].bitcast(F32),
                        mean_ps[:], op=ALU.subtract)
                    nc.vector.tensor_tensor(dst[:, c], cen[:], rstd[:],
                                            op=ALU.mult)

            def proj_ln(w_groups, ws_idx, rstd, mrow, dst, pre=None):
                """dst[:, j] = rstd * (W @ x - m*rowsum(W)) per chunk, bf16."""
                for g in range(2):
                    if pre and g in pre:
                        wt = pre.pop(g)
                    else:
                        wt = wpool.tile([P, DCH, N], BF16, tag="wt", name="wt")
                        nc.sync.dma_start(wt[:, 0:4], w_groups[g][:, 0:4])
                        nc.sync.dma_start(wt[:, 4:8], w_groups[g][:, 4:8])
                    for jj in range(4):
                        j = g * 4 + jj
                        ps = psum.tile([P, N], F32, tag="mm", bufs=2, name="ps_mm")
                        for k in range(DCH):
                            nc.tensor.matmul(ps[:], wt[:, k, jj * P:(jj + 1) * P],
                                             xTb[:, k], start=(k == 0), stop=False,
                                             skip_group_check=True)
                        nc.tensor.matmul(ps[:], wsT[0:1, ws_idx, j * P:(j + 1) * P],
                                         mrow[:], start=False, stop=True,
                                         skip_group_check=True)
                        nc.vector.tensor_tensor(dst[:, j], ps[:], rstd[:],
                                                op=ALU.mult)

            def proj(rhs, w_groups, evict):
                """out[j] = sum_k W[k, j-chunk].T @ rhs[k]."""
                for g in range(2):
                    wt = wpool.tile([P, DCH, N], BF16, tag="wt", name="wt")
                    nc.sync.dma_start(wt[:, 0:4], w_groups[g][:, 0:4])
                    nc.sync.dma_start(wt[:, 4:8], w_groups[g][:, 4:8])
                    for jj in range(4):
                        j = g * 4 + jj
                        ps = psum.tile([P, N], F32, tag="mm", bufs=2, name="ps_mm")
                        for k in range(DCH):
                            nc.tensor.matmul(ps[:], wt[:, k, jj * P:(jj + 1) * P],
                                             rhs[:, k], start=(k == 0),
                                             stop=(k == DCH - 1))
                        evict(j, ps)

            def v_proj(w_groups):
                """V_nat[tk, dv] -> Vst[:, t, h, :64] slices (from hT)."""
                for g in range(2):  # dv halves (heads g*8..g*8+7)
                    wt = wpool.tile([P, DCH, N], BF16, tag="wt", name="wt")
                    nc.sync.dma_start(wt[:], w_groups[g])
                    for t in range(MCH):
                        ps = psum.tile([P, N], F32, tag="mm", bufs=2, name="ps_v")
                        for k in range(DCH):
                            nc.tensor.matmul(ps[:], hT[:, k, t * P:(t + 1) * P],
                                             wt[:, k], start=(k == 0),
                                             stop=(k == DCH - 1))
                        nc.scalar.activation(
                            Vst[:, t, g * 8:(g + 1) * 8, NONES:NONES + 64],
                            ps[:].rearrange("p (h d) -> p h d", d=64), AF.Copy)

            def sdp_scores(q_t, k_t, h, is_self):
                """Scores_T -> exp for one head; ctx is emitted one head later
                (software pipeline) so the exp latency never stalls the PE."""
                hc, off = h // 2, 64 * (h % 2)
                exps = []
                for c in range(MCH):
                    cs = c * P if is_self else 0
                    sc = psum.tile([P, N], F32, tag="sc", bufs=2, name="sc")
                    nc.tensor.matmul(sc[:, cs:],
                                     k_t[off:off + 64, hc, c * P:(c + 1) * P],
                                     q_t[off:off + 64, hc, cs:],
                                     start=True, stop=not is_self,
                                     skip_group_check=True)
                    if is_self:
                        # causal mask for the diagonal block, in PSUM
                        nc.tensor.matmul(sc[:, cs:cs + P], maskT[:], identb[:],
                                         start=False, stop=True,
                                         skip_group_check=True)
                    ex = spool.tile([P, N], BF16, tag="exp", bufs=16, name="ex")
                    nc.scalar.activation(ex[:, cs:], sc[:, cs:], AF.Exp,
                                         scale=INV_SQRT_DK)
                    exps.append(ex)
                return exps

            def sdp_ctx(h, exps, is_self):
                hc, off = h // 2, 64 * (h % 2)
                ctx = psum.tile([P, N], F32, tag="ctx", bufs=2, name="ctx")
                for c in range(MCH):
                    cs = c * P if is_self else 0
                    nc.tensor.matmul(ctx[:, cs:], Vst[:, c, h, :],
                                     exps[c][:, cs:], start=(c == 0),
                                     stop=(c == MCH - 1), skip_group_check=True)
                # ones-columns come FIRST in Vst so the denominators land at
                # PSUM partition base 0: reciprocal_approx_fast (custom DVE
                # op) mishandles nonzero partition offsets on PSUM inputs on
                # hardware (reads base 0 regardless; CoreSim models it right).
                rec = spool.tile([NONES, N], F32, tag="rec", name="rec")
                nc.vector.reciprocal_approx_fast(rec[:], ctx[0:NONES, :])
                nc.vector.tensor_tensor(ctxT[off:off + 64, hc, :],
                                        ctx[NONES:NONES + 64, :],
                                        rec[:], op=ALU.mult)

            def attention(q_t, k_t, is_self, wo_groups, out_evict, fillers=None):
                """`fillers` maps head-index -> callables emitting independent
                PE work (the next attention's K/V projections)."""
                pend = None
                for h in range(H):
                    if fillers:
                        for cb in fillers.get(h, ()):
                            cb()
                    exps = sdp_scores(q_t, k_t, h, is_self)
                    if pend is not None:
                        sdp_ctx(pend, pend_exps, is_self)
                    pend, pend_exps = h, exps
                sdp_ctx(pend, pend_exps, is_self)
                if fillers:
                    for cb in fillers.get(H, ()):
                        cb()
                proj(ctxT, wo_groups, out_evict)

            for l in range(L):
                def wgrp(a, i):
                    return [wp[l, a, i, g] for g in range(2)]

                # ---- self-attention ----
                nc.sync.dma_start(wsT[:], wsums[l])
                nc.sync.dma_start(ws1T[:], ws1s[l])
                rstd1, mrow1 = ln_chain(mean_ps, msq_ps)
                proj_ln(wgrp(0, 0), 0, rstd1, mrow1, qT,
                        pre={0: _pre.pop(("q", 0)), 1: _pre.pop(("q", 1))})
                proj_ln(wgrp(0, 1), 1, rstd1, mrow1, kT,
                        pre={0: _pre.pop(("k", 0))})
                ln_apply(hT, xT, mean_ps, rstd1)
                v_proj(wgrp(0, 2))
                if DBG and l == 0:
                    nc.sync.dma_start(dbgb[2], qT[:, 0])
                    nc.sync.dma_start(dbgb[3], kT[:, 0])
                    nc.sync.dma_start(dbgb[4], hT[:, 0])
                    nc.sync.dma_start(dbgb[6][:, 0:128], Vst[:, 0, 0, :])
                    nc.sync.dma_start(dbgb[7][:, 0:128], Vst[:, 0, 8, :])

                # LN2 stats ride the self-attn residual evictions
                mean_ps2 = psum.tile([P, N], F32, tag="st", bufs=2, name="mean2")
                msq_ps2 = psum.tile([P, N], F32, tag="st", bufs=2, name="msq2")

                def ev_self_o(j, ps):
                    nc.vector.tensor_tensor(xT[:, j], ps[:], xT[:, j].bitcast(F32),
                                            op=ALU.add)
                    nc.scalar.activation(xTb[:, j], xT[:, j].bitcast(F32),
                                         AF.Copy)
                    nc.scalar.activation(sqT[:, j], xTb[:, j], AF.Square)
                    stats_mm(mean_ps2, msq_ps2, j, xTb)

                # cross-attn K/V projections fill the self-attention SDP
                ck_state = {}
                wkx, wvx = wgrp(1, 1), wgrp(1, 2)

                def ck_dma(g):
                    def f():
                        wt = wpool.tile([P, DCH, N], BF16, tag="wt", name="wt")
                        nc.sync.dma_start(wt[:, 0:4], wkx[g][:, 0:4])
                        nc.sync.dma_start(wt[:, 4:8], wkx[g][:, 4:8])
                        ck_state["k"] = wt
                    return f

                def ck_chunk(g, jj):
                    def f():
                        wt = ck_state["k"]
                        j = g * 4 + jj
                        ps = psum.tile([P, N], F32, tag="mm", bufs=2, name="ps_ck")
                        for k in range(DCH):
                            nc.tensor.matmul(ps[:], wt[:, k, jj * P:(jj + 1) * P],
                                             encT[:, k], start=(k == 0),
                                             stop=(k == DCH - 1))
                        nc.vector.tensor_copy(kT[:, j], ps[:])
                    return f

                def cv_dma(g):
                    def f():
                        wt = wpool.tile([P, DCH, N], BF16, tag="wt", name="wt")
                        nc.sync.dma_start(wt[:], wvx[g])
                        ck_state["v"] = wt
                    return f

                def cv_chunk(g, t):
                    def f():
                        wt = ck_state["v"]
                        ps = psum.tile([P, N], F32, tag="mm", bufs=2, name="ps_cv")
                        for k in range(DCH):
                            nc.tensor.matmul(ps[:], encT[:, k, t * P:(t + 1) * P],
                                             wt[:, k], start=(k == 0),
                                             stop=(k == DCH - 1))
                        nc.vector.tensor_copy(
                            Vst[:, t, g * 8:(g + 1) * 8, NONES:NONES + 64],
                            ps[:].rearrange("p (h d) -> p h d", d=64))
                    return f

                fillers = {
                    0: [ck_dma(0)],
                    2: [ck_chunk(0, 0)], 4: [ck_chunk(0, 1)],
                    6: [ck_chunk(0, 2)], 8: [ck_chunk(0, 3), ck_dma(1)],
                    10: [ck_chunk(1, 0), cv_dma(0)],
                    12: [ck_chunk(1, 1), cv_chunk(0, 0)],
                    13: [cv_chunk(0, 1)],
                    14: [ck_chunk(1, 2), cv_chunk(0, 2)],
                    15: [cv_chunk(0, 3)],
                    16: [ck_chunk(1, 3), cv_dma(1),
                         cv_chunk(1, 0), cv_chunk(1, 1),
                         cv_chunk(1, 2), cv_chunk(1, 3)],
                }
                attention(qT, kT, True, wgrp(0, 3), ev_self_o, fillers=fillers)
                if DBG and l == 0:
                    nc.sync.dma_start(dbgf[4], xT[:, 0].bitcast(F32))
                    nc.sync.dma_start(dbgb[8], ctxT[:, 0])

                # ---- cross-attention ----
                # Q-projection runs one chunk ahead of its SDP head pairs.
                rstd2, mrow2 = ln_chain(mean_ps2, msq_ps2)
                wqx = wgrp(1, 0)

                def qx_chunk(g, jj, wt):
                    j = g * 4 + jj
                    ps = psum.tile([P, N], F32, tag="mm", bufs=2, name="ps_cq")
                    for k in range(DCH):
                        nc.tensor.matmul(ps[:], wt[:, k, jj * P:(jj + 1) * P],
                                         xTb[:, k], start=(k == 0), stop=False,
                                         skip_group_check=True)
                    nc.tensor.matmul(ps[:], wsT[0:1, 2, j * P:(j + 1) * P],
                                     mrow2[:], start=False, stop=True,
                                     skip_group_check=True)
                    nc.vector.tensor_tensor(qT[:, j], ps[:], rstd2[:],
                                            op=ALU.mult)

                wts = {}
                for g in range(2):
                    wt = wpool.tile([P, DCH, N], BF16, tag="wt", name="wt")
                    nc.sync.dma_start(wt[:, 0:4], wqx[g][:, 0:4])
                    nc.sync.dma_start(wt[:, 4:8], wqx[g][:, 4:8])
                    wts[g] = wt
                xexps = {}
                for j in range(DCH + 1):
                    if j < DCH:
                        qx_chunk(j // 4, j % 4, wts[j // 4])
                    if j >= 1:
                        for h in (2 * (j - 1), 2 * (j - 1) + 1):
                            xexps[h] = sdp_scores(qT, kT, h, False)
                    if j >= 2:
                        for h in (2 * (j - 2), 2 * (j - 2) + 1):
                            sdp_ctx(h, xexps.pop(h), False)
                for h in (14, 15):
                    sdp_ctx(h, xexps.pop(h), False)

                # prefetch first FFN weight tiles during the cross O-proj
                w1t = {0: fpool.tile([P, DCH, N], BF16, tag="w1", name="w1t")}
                nc.sync.dma_start(w1t[0][:], w1p[l, 0])
                w2t = {0: fpool.tile([P, DCH, 4, P], BF16, tag="w2", name="w2t")}
                nc.sync.dma_start(w2t[0][:], w2p[l, 0])

                # LN3 stats (over x+c, kept in qT) ride the cross evictions
                mean_ps3 = psum.tile([P, N], F32, tag="st", bufs=2, name="mean3")
                msq_ps3 = psum.tile([P, N], F32, tag="st", bufs=2, name="msq3")

                def ev_cross_o(j, ps):
                    # qT <- x + c (for LN3); xT <- c (reference residual quirk)
                    nc.vector.tensor_tensor(qT[:, j], ps[:], xT[:, j].bitcast(F32),
                                            op=ALU.add)
                    nc.scalar.activation(xT[:, j].bitcast(F32), ps[:], AF.Copy)
                    nc.scalar.activation(sqT[:, j], qT[:, j], AF.Square)
                    stats_mm(mean_ps3, msq_ps3, j, qT)
                proj(ctxT, wgrp(1, 3), ev_cross_o)
                if DBG and l == 0:
                    nc.sync.dma_start(dbgb[9], qT[:, 0])
                    nc.sync.dma_start(dbgf[5], xT[:, 0].bitcast(F32))

                # ---- FFN (8 f-eighths, software-pipelined) ----
                # FFN1 uses the LN-fixup scheme on sT (in qT) directly:
                # relu(rstd * (W1 @ sT - m * rowsum(W1)))
                rstd3, mrow3 = ln_chain(mean_ps3, msq_ps3)

                if l + 1 < L:
                    mean_psN = psum.tile([P, N], F32, tag="st", bufs=2, name="meanN")
                    msq_psN = psum.tile([P, N], F32, tag="st", bufs=2, name="msqN")

                u8s = {}

                def ffn1(e8):
                    u8 = spool.tile([P, 4, N], BF16, tag="uT", name="u8")
                    wt = w1t.pop(e8)
                    for jj in range(4):
                        f = e8 * 4 + jj
                        ps = psum.tile([P, N], F32, tag="mm", bufs=2, name="ps_f1")
                        for k in range(DCH):
                            nc.tensor.matmul(ps[:], wt[:, k, jj * P:(jj + 1) * P],
                                             qT[:, k], start=(k == 0), stop=False,
                                             skip_group_check=True)
                        nc.tensor.matmul(ps[:], ws1T[0:1, f * P:(f + 1) * P],
                                         mrow3[:], start=False, stop=True,
                                         skip_group_check=True)
                        t1 = spool.tile([P, N], F32, tag="cen", name="t1")
                        nc.vector.tensor_tensor(t1[:], ps[:], rstd3[:],
                                                op=ALU.mult)
                        nc.scalar.activation(u8[:, jj], t1[:], AF.Relu)
                    u8s[e8] = u8

                def ffn2(e8):
                    u8 = u8s.pop(e8)
                    wt2 = w2t.pop(e8)
                    for j in range(DCH):
                        ps = psum.tile([P, N], F32, tag="mm", bufs=2, name="ps_f2")
                        for k in range(4):
                            nc.tensor.matmul(ps[:], wt2[:, j, k], u8[:, k],
                                             start=(k == 0), stop=(k == 3))
                        # for e8 == 0 this is x = cross_out + ffn_part0
                        # (xT already holds c, the reference residual quirk)
                        nc.vector.tensor_tensor(xT[:, j], ps[:],
                                                xT[:, j].bitcast(F32),
                                                op=ALU.add)
                        if e8 == 7 and l + 1 < L:
                            nc.scalar.activation(xTb[:, j],
                                                 xT[:, j].bitcast(F32), AF.Copy)
                            nc.scalar.activation(sqT[:, j], xTb[:, j], AF.Square)
                            stats_mm(mean_psN, msq_psN, j, xTb)
                        if e8 == 7 and l + 1 == L:
                            for m in range(MCH):
                                pst = psum.tile([P, N], F32, tag="sc", bufs=2,
                                                name="pst")
                                nc.tensor.transpose(
                                    pst[:, 0:P],
                                    xT[:, j, m * P:(m + 1) * P].bitcast(F32),
                                    ident[:])
                                tsb = spool.tile([P, P], F32, tag="osb", bufs=2,
                                                 name="tsb")
                                nc.scalar.activation(tsb[:], pst[:, 0:P], AF.Copy)
                                nc.sync.dma_start(
                                    out[m * P:(m + 1) * P, j * P:(j + 1) * P],
                                    tsb[:])

                w1t[1] = fpool.tile([P, DCH, N], BF16, tag="w1", name="w1t")
                nc.sync.dma_start(w1t[1][:], w1p[l, 1])
                ffn1(0)
                for e8 in range(8):
                    if e8 == 3 and l + 1 < L:
                        _pre.update(preload_qk(l + 1))
                    if e8 + 1 < 8:
                        w2t[e8 + 1] = fpool.tile([P, DCH, 4, P], BF16,
                                                 tag="w2", name="w2t")
                        nc.sync.dma_start(w2t[e8 + 1][:], w2p[l, e8 + 1])
                        if e8 + 2 < 8:
                            w1t[e8 + 2] = fpool.tile([P, DCH, N], BF16,
                                                     tag="w1", name="w1t")
                            nc.sync.dma_start(w1t[e8 + 2][:], w1p[l, e8 + 2])
                        ffn1(e8 + 1)
                    ffn2(e8)
                if DBG:
                    nc.sync.dma_start(dbgf[6 + l], xT[:, 0].bitcast(F32))

                if l + 1 < L:
                    mean_ps, msq_ps = mean_psN, msq_psN

            work.release()

    nc.compile()
    return nc


# ----------------------------------------------------------------------------
# host-side packing
# ----------------------------------------------------------------------------

def _pack_T(aT):
    """[1024, C] (feature-major) -> tile image [128, 8, C]."""
    d, c = aT.shape
    return np.ascontiguousarray(aT.reshape(DCH, P, c).transpose(1, 0, 2))


def _pack_proj(w):
    """w [Dout, Din] (as in y = x @ w.T) -> [2, 128, 8, 512] group tile images."""
    wT = w.T  # [Din, Dout]
    return np.stack([_pack_T(wT[:, g * N:(g + 1) * N]) for g in range(2)])


def _prep(inputs):
    BH = ml_dtypes.bfloat16
    dec_inputs = np.asarray(inputs["dec_inputs"])
    self_mask = np.asarray(inputs["self_mask"])
    enc_output = np.asarray(inputs["enc_output"], dtype=np.float32)
    encoder_mask = np.asarray(inputs["encoder_mask"])
    embed = np.asarray(inputs["embed"], dtype=np.float32)
    pe = np.asarray(inputs["pe"], dtype=np.float32)
    wq, wk, wv, wo = (np.asarray(inputs[k], np.float32) for k in ("wq", "wk", "wv", "wo"))
    w1, w2 = np.asarray(inputs["ffn_w1"], np.float32), np.asarray(inputs["ffn_w2"], np.float32)

    # structural assumptions baked into the kernel
    causal_ref = np.triu(np.ones((LD, LD), bool), k=1)
    assert all(np.array_equal(self_mask[b], causal_ref) for b in range(B)), \
        "kernel assumes causal self mask"
    assert not encoder_mask.any(), "kernel assumes no encoder mask"
    for k in ("bq", "bk", "bv", "bo", "ffn_b1", "ffn_b2", "ln_b"):
        assert not np.asarray(inputs[k]).any(), f"kernel assumes zero {k}"
    assert np.all(np.asarray(inputs["ln_g"]) == 1.0), "kernel assumes unit ln gains"

    # shared (weight) arrays
    wp = np.empty((L, 2, 4, 2, P, DCH, N), BH)
    for l in range(L):
        for a in range(2):
            for i, w in enumerate((wq, wk, wv, wo)):
                wp[l, a, i] = _pack_proj(w[l, a]).astype(BH)
    w1p = np.empty((L, 8, P, DCH, N), BH)
    w2p = np.empty((L, 8, P, DCH, 4, P), BH)
    for l in range(L):
        w1T = w1[l].T  # [1024, 4096]
        for g in range(8):
            w1p[l, g] = _pack_T(w1T[:, g * N:(g + 1) * N]).astype(BH)
        w2T = w2[l].T  # [4096, 1024]
        blk = w2T.reshape(8, 4, P, DCH, P)
        w2p[l] = np.ascontiguousarray(blk.transpose(0, 2, 3, 1, 4)).astype(BH)

    # negated row-sums for the LN fixup matmuls: q_self, k_self, q_cross
    wsums = np.empty((L, 1, 3, D), np.float32)
    ws1s = np.empty((L, 1, F), np.float32)
    for l in range(L):
        wsums[l, 0, 0] = -wq[l, 0].sum(axis=1)
        wsums[l, 0, 1] = -wk[l, 0].sum(axis=1)
        wsums[l, 0, 2] = -wq[l, 1].sum(axis=1)
        ws1s[l, 0] = -w1[l].sum(axis=1)

    pep = _pack_T(pe.T).astype(BH)
    maskp = (NEG * np.triu(np.ones((P, P), np.float32), 1)).astype(BH)
    jscaleb = np.full((P, P), 1.0 / D, BH)
    vones = np.ones((P, MCH, H, NONES), BH)

    shared = dict(wp=wp, w1p=w1p, w2p=w2p, pep=pep, maskp=maskp,
                  jscaleb=jscaleb, vones=vones,
                  wsums=wsums.astype(BH), ws1s=ws1s.astype(BH))
    in_maps = []
    for b in range(B):
        x0 = embed[dec_inputs[b]]          # [512, 1024]
        m = dict(shared)
        m["x0p"] = _pack_T(np.ascontiguousarray(x0.T)).astype(BH)
        m["encp"] = _pack_T(np.ascontiguousarray(enc_output[b].T)).astype(BH)
        in_maps.append(m)
    return in_maps


def kernel(**inputs):
    if "nc" not in _CACHE:
        _CACHE["nc"] = _build_nc()
    nc = _CACHE["nc"]
    in_maps = _prep(inputs)

    from concourse import bass_utils
    trace = bool(int(os.environ.get("DECODER_TRACE", "0")))
    res = bass_utils.run_bass_kernel_spmd(
        nc, in_maps, core_ids=list(range(B)), trace=trace)
    _CACHE["last_result"] = res
    return np.stack([res.results[b]["out"] for b in range(B)]).astype(np.float32)
